# revision 1
# baseline (speedup 1.0000x reference)
"""Trainium2 Bass kernel for nn_MultiHeadedAttention_19713899889501.

Strategy: pure data-parallel over batch (B=8 -> 8 NeuronCores), no
collectives.  Per core, one batch element:

  qagg[t] = sum_{j<5} q[t+j]                  (unweighted window sum)
  kagg[t] = sum_j softmax_j(<k[t+4],k[t+j]>/sqrt(D)) k[t+j]
  vlin    = v[4:] @ W0 + b0
  out     = MHA(qagg, kagg, vlin) @ Wout + bout

Device layout is fully transposed (d on partitions, seq on free dim):
  - qaggT via DVE shift-add tree on host-pre-transposed qT
  - local scores via DVE products + PE ones-column (M=1) reduces, the 5
    lags laid side by side in the free dim of one PSUM partition
  - softmax normalization of the local weights is folded into a kagg
    pre-scale (1/wsum); 1/sqrt(DK) goes into the ACT exp scale immediate
  - QK^T head-paired: two K=64 matmuls on disjoint PE row groups run
    concurrently (rows 0-63 = even head, 64-127 = odd head)
  - exp on ScalarE over (128, 1024) two-bank PSUM reads, bf16 out; the
    s-loop is software-pipelined (QK of s-tile st+1 issues before att@V
    of st) so the in-order PE never stalls behind the exp
  - att@V with lhsT = [v_head 64 cols | ones 64 cols] so the softmax
    denominators come out replicated across 64 partitions for free
  - kagg for chunks 1..3 and the output linear are scheduled to overlap
    the SDPA loop (chunk-0 kagg unblocks the first head pair early)
  - output linear with bias via K=1 ones-row matmul

The t/s grid is padded 2044 -> 2048; padded key positions are nulled by
zeroing their vlin rows (including the ones column); padded t rows are
simply not written back.
"""
import sys

if "/opt/trn_rl_repo" not in sys.path:
    sys.path.insert(0, "/opt/trn_rl_repo")

import numpy as np
import ml_dtypes

import concourse.bass as bass
import concourse.tile as tile
import concourse.mybir as mybir
from concourse import bacc
from concourse.bass_utils import run_bass_kernel_spmd

BF16 = ml_dtypes.bfloat16
F32 = mybir.dt.float32
BF = mybir.dt.bfloat16
AF = mybir.ActivationFunctionType

B, S, D, H, L = 8, 2048, 512, 8, 5
DK = D // H           # 64
SP = S - L + 1        # 2044 true output positions
SPP = 2048            # padded t/s grid
SQ = S + 8            # padded qT/kT width (2056)
NCH = 4               # d chunks of 128
NT = 4                # t chunks of 512
NS = 16               # s tiles of 128
N_CORES = 8

_PROGRAM = None


def _build_core_program():
    nc = bacc.Bacc("TRN2", target_bir_lowering=False, debug=False)

    qT = nc.dram_tensor("qT", [D, SQ], BF, kind="ExternalInput").ap()
    kT = nc.dram_tensor("kT", [D, SQ], BF, kind="ExternalInput").ap()
    vT = nc.dram_tensor("vT", [D, SPP], BF, kind="ExternalInput").ap()
    W0 = nc.dram_tensor("W0", [D, D], BF, kind="ExternalInput").ap()
    Wout = nc.dram_tensor("Wout", [D, D], BF, kind="ExternalInput").ap()
    b0 = nc.dram_tensor("b0", [1, D], BF, kind="ExternalInput").ap()
    bout = nc.dram_tensor("bout", [1, D], BF, kind="ExternalInput").ap()
    zpad = nc.dram_tensor("zpad", [4, 1024], BF, kind="ExternalInput").ap()
    out = nc.dram_tensor("out", [SP, D], F32, kind="ExternalOutput").ap()

    with tile.TileContext(nc) as tc:
        _build(tc, qT, kT, vT, W0, Wout, b0, bout, zpad, out)
    nc.compile()
    return nc


def _build(tc, qT, kT, vT, W0, Wout, b0, bout, zpad, out):
    nc = tc.nc
    from contextlib import ExitStack

    inv_sqrt_d = float(1.0 / np.sqrt(np.float32(D)))
    inv_sqrt_dk = float(1.0 / np.sqrt(np.float32(DK)))

    with ExitStack() as ctx:
        pers = ctx.enter_context(tc.tile_pool(name="pers", bufs=1))

        # ---------------- constants / weights ----------------
        ones_row = pers.tile([1, 128], BF, tag="ones_row")
        nc.vector.memset(ones_row[:], 1.0)
        ones_col = pers.tile([128, 1], BF, tag="ones_col")
        nc.vector.memset(ones_col[:], 1.0)

        b0_sb = pers.tile([1, D], BF, tag="b0")
        nc.sync.dma_start(b0_sb[:], b0[:])
        bout_sb = pers.tile([1, D], BF, tag="bout")
        nc.sync.dma_start(bout_sb[:], bout[:])

        kTb = []
        for c in range(NCH):
            t = pers.tile([128, SQ], BF, tag=f"kTb{c}")
            nc.sync.dma_start(t[:], kT[c * 128:(c + 1) * 128, :])
            kTb.append(t)
        W0b = []
        Woutb = []
        for c in range(NCH):
            t = pers.tile([128, D], BF, tag=f"W0b{c}")
            nc.sync.dma_start(t[:], W0[c * 128:(c + 1) * 128, :])
            W0b.append(t)
            t = pers.tile([128, D], BF, tag=f"Woutb{c}")
            nc.sync.dma_start(t[:], Wout[c * 128:(c + 1) * 128, :])
            Woutb.append(t)

        # persistent results of the prefix
        qaggT = []
        kaggT = []
        vlin_sb = []
        XhatT = []
        for c in range(NCH):
            t = pers.tile([128, SPP], BF, tag=f"qaggT{c}")
            qaggT.append(t)
            t = pers.tile([128, SPP], BF, tag=f"kaggT{c}")
            kaggT.append(t)
            t = pers.tile([128, SPP], BF, tag=f"XhatT{c}")
            XhatT.append(t)
        # per head h: cols [128h:128h+64] = v head cols, [128h+64:128h+128] = ones
        for st in range(NS):
            t = pers.tile([128, 1024], BF, tag=f"vlin{st}")
            vlin_sb.append(t)

        # SDPA-era SBUF pools open FIRST so their space does not reuse
        # prefix-pool space (which would chain SDPA startup to prefix tails).
        pap = ctx.enter_context(tc.tile_pool(name="pap", bufs=3))
        rxp = ctx.enter_context(tc.tile_pool(name="rxp", bufs=2))
        osbp = ctx.enter_context(tc.tile_pool(name="osbp", bufs=2))
        edram = ctx.enter_context(tc.tile_pool(name="edram", bufs=1, space="DRAM"))

        # chunk-0 qagg tree first: its qT DMA precedes the 4MB vT
        # transfers, and it gates the first SDPA head pair (vlin has slack).
        def qagg_tree(c, treep):
            x = treep.tile([128, SQ], BF, tag="qT_in")
            nc.sync.dma_start(x[:], qT[c * 128:(c + 1) * 128, :])
            s1 = treep.tile([128, 2052], BF, tag="tree1")
            nc.vector.tensor_add(s1[:], x[:, 0:2052], x[:, 1:2053])
            s2 = treep.tile([128, SPP], BF, tag="tree2")
            nc.vector.tensor_add(s2[:], s1[:, 0:SPP], s1[:, 2:2 + SPP])
            nc.vector.tensor_add(qaggT[c][:], s2[:], x[:, 4:4 + SPP])

        treep = ctx.enter_context(tc.tile_pool(name="treep", bufs=1))
        qagg_tree(0, treep)

        # ================ prefix phase A: vlin ============
        with ExitStack() as pre:
            vtp = pre.enter_context(tc.tile_pool(name="vtp", bufs=1))
            pre_ps = pre.enter_context(
                tc.tile_pool(name="pre_ps", bufs=2, space="PSUM"))

            vtbs = []
            for c in range(NCH):
                vtb = vtp.tile([128, SPP], BF, tag=f"vTb{c}")
                nc.sync.dma_start(vtb[:], vT[c * 128:(c + 1) * 128, :])
                vtbs.append(vtb)
            for st in range(NS):
                ps = pre_ps.tile([128, 512], F32, tag="vlin_ps")
                for c in range(NCH):
                    nc.tensor.matmul(
                        ps[:], vtbs[c][:, st * 128:(st + 1) * 128], W0b[c][:],
                        start=(c == 0), stop=False,
                    )
                nc.tensor.matmul(ps[:], ones_row[:], b0_sb[:],
                                 start=False, stop=True)
                vre = vlin_sb[st].rearrange("p (h u) -> p h u", u=128)
                psr = ps.rearrange("p (h u) -> p h u", u=64)
                nc.scalar.activation(vre[:, :, 0:64], psr[:], AF.Copy)
                nc.gpsimd.memset(vre[:, :, 64:128], 1.0)
                if st == NS - 1:
                    # zero padded key rows (engine ops can't address base 124;
                    # DMA is address-based and can)
                    nc.sync.dma_start(vlin_sb[st][124:128, :], zpad[:])


        # ================ prefix phase C: local scores + kagg ============
        # sweep 1: products -> scr4 -> e4 -> wsum/recip -> broadcasts ->
        #          kagg for chunk 0 only (unblocks the first SDPA head pair);
        #          e4 rows spill to DRAM for sweep 2.
        # sweep 2: re-broadcast e/recip per quarter, kagg for chunks 1..3
        #          (overlaps the SDPA main loop on DVE/GpSimd).
        with ExitStack() as pre:
            prodp = pre.enter_context(tc.tile_pool(name="prodp", bufs=2))
            e4p = pre.enter_context(tc.tile_pool(name="e4p", bufs=2))
            wrowp = pre.enter_context(tc.tile_pool(name="wrowp", bufs=1))
            ebp = pre.enter_context(tc.tile_pool(name="ebp", bufs=1))
            kwp = pre.enter_context(tc.tile_pool(name="kwp", bufs=3))
            scr_ps = pre.enter_context(
                tc.tile_pool(name="scr_ps", bufs=1, space="PSUM"))

            e4_dram = []
            for t4 in range(NT):
                ed = edram.tile([1, 5 * 512], F32, tag=f"e4d{t4}")
                e4_dram.append(ed)
            rrow_keep = []

            def bcast_quarter(e4t, rrowt):
                ebs = []
                for j in range(L):
                    eb = ebp.tile([128, 512], F32, tag=f"eb{j}")
                    nc.gpsimd.partition_broadcast(
                        eb[:], e4t[:, j * 512:(j + 1) * 512])
                    ebs.append(eb)
                recipb = ebp.tile([128, 512], F32, tag="recipb")
                nc.gpsimd.partition_broadcast(recipb[:], rrowt[:])
                return ebs, recipb

            def kagg_quarter(c, t4, ebs, recipb):
                sl = slice(t4 * 512, (t4 + 1) * 512)
                acc = kwp.tile([128, 512], F32, tag="kacc")
                nc.vector.tensor_mul(
                    acc[:], kTb[c][:, t4 * 512:t4 * 512 + 512], ebs[0][:])
                for j in range(1, L):
                    term = kwp.tile([128, 512], F32, tag="kterm")
                    nc.vector.tensor_mul(
                        term[:], kTb[c][:, t4 * 512 + j:t4 * 512 + j + 512],
                        ebs[j][:])
                    acc2 = kwp.tile([128, 512], F32, tag="kacc")
                    nc.vector.tensor_add(acc2[:], acc[:], term[:])
                    acc = acc2
                nc.vector.tensor_mul(kaggT[c][:, sl], acc[:], recipb[:])

            # ---- sweep 1 ----
            for t4 in range(NT):
                scr4 = scr_ps.tile([1, 5 * 512], F32, tag="scr4")
                for j in range(L):
                    for c in range(NCH):
                        p = prodp.tile([128, 512], BF, tag="prod")
                        nc.vector.tensor_mul(
                            p[:],
                            kTb[c][:, t4 * 512 + 4:t4 * 512 + 4 + 512],
                            kTb[c][:, t4 * 512 + j:t4 * 512 + j + 512])
                        nc.tensor.matmul(
                            scr4[:, j * 512:(j + 1) * 512],
                            ones_col[:], p[:],
                            start=(c == 0), stop=(c == NCH - 1),
                        )
                e4 = e4p.tile([1, 5 * 512], F32, tag="e4")
                nc.scalar.activation(e4[:], scr4[:], AF.Exp, scale=inv_sqrt_d)
                nc.sync.dma_start(e4_dram[t4][:], e4[:])
                w1 = wrowp.tile([1, 512], F32, tag="w1")
                nc.vector.tensor_add(w1[:], e4[:, 0:512], e4[:, 512:1024])
                w2 = wrowp.tile([1, 512], F32, tag="w2")
                nc.vector.tensor_add(w2[:], e4[:, 1024:1536], e4[:, 1536:2048])
                w3 = wrowp.tile([1, 512], F32, tag="w3")
                nc.vector.tensor_add(w3[:], w1[:], w2[:])
                wsum = wrowp.tile([1, 512], F32, tag="wsum")
                nc.vector.tensor_add(wsum[:], w3[:], e4[:, 2048:2560])
                rrow = pers.tile([1, 512], F32, tag=f"rrow{t4}")
                nc.vector.reciprocal(rrow[:], wsum[:])
                rrow_keep.append(rrow)

                ebs, recipb = bcast_quarter(e4, rrow)
                kagg_quarter(0, t4, ebs, recipb)

            for _c in range(1, NCH):
                qagg_tree(_c, treep)

            # ---- sweep 2 (overlaps SDPA) ----
            for t4 in range(NT):
                e4 = e4p.tile([1, 5 * 512], F32, tag="e4")
                nc.sync.dma_start(e4[:], e4_dram[t4][:])
                ebs, recipb = bcast_quarter(e4, rrow_keep[t4])
                for c in range(1, NCH):
                    kagg_quarter(c, t4, ebs, recipb)

        # ======================== SDPA main loop ============================
        with ExitStack() as main:
            qk_ps = main.enter_context(
                tc.tile_pool(name="qk_ps", bufs=2, space="PSUM"))
            x_ps_pool = main.enter_context(
                tc.tile_pool(name="x_ps", bufs=2, space="PSUM"))

            def outlin_block(tb):
                o_ps = x_ps_pool.tile([128, 512], F32, tag="xA")
                for c in range(NCH):
                    nc.tensor.matmul(
                        o_ps[:], XhatT[c][:, tb * 128:(tb + 1) * 128], Woutb[c][:],
                        start=(c == 0), stop=False,
                    )
                nc.tensor.matmul(o_ps[:], ones_row[:], bout_sb[:],
                                 start=False, stop=True)
                o_sb = osbp.tile([128, 512], F32, tag="o_sb")
                nc.scalar.activation(o_sb[:], o_ps[:], AF.Copy)
                rows = 128 if tb < NS - 1 else SP - 128 * (NS - 1)
                nc.sync.dma_start(out[tb * 128: tb * 128 + rows, :],
                                  o_sb[0:rows, :])

            for pair in range(H // 2):
                c = pair          # chunk c holds heads 2c (rows 0:64), 2c+1 (64:128)
                hA, hB = 2 * pair, 2 * pair + 1
                for tcx in range(NT):
                    tsl = slice(tcx * 512, (tcx + 1) * 512)
                    xA = x_ps_pool.tile([128, 512], F32, tag="xA")
                    xB = x_ps_pool.tile([128, 512], F32, tag="xB")
                    pending = None
                    # software pipeline: QK(st) is emitted before attV(st-1)
                    # so the PE never stalls behind the exp of the current
                    # s-tile before issuing the next QK.
                    for st in range(NS):
                        ssl = slice(st * 128, (st + 1) * 128)
                        p_ps = qk_ps.tile([128, 1024], F32, tag="p_ps")
                        nc.tensor.matmul(
                            p_ps[:, 0:512],
                            kaggT[c][0:64, ssl], qaggT[c][0:64, tsl],
                            start=True, stop=True,
                        )
                        nc.tensor.matmul(
                            p_ps[:, 512:1024],
                            kaggT[c][64:128, ssl], qaggT[c][64:128, tsl],
                            start=True, stop=True,
                        )
                        pa = pap.tile([128, 1024], BF, tag="pa")
                        nc.scalar.activation(pa[:], p_ps[:], AF.Exp,
                                             scale=inv_sqrt_dk)
                        if pending is not None:
                            ppa, pst = pending
                            nc.tensor.matmul(
                                xA[:], vlin_sb[pst][:, hA * 128:(hA + 1) * 128],
                                ppa[:, 0:512],
                                start=(pst == 0), stop=False,
                            )
                            nc.tensor.matmul(
                                xB[:], vlin_sb[pst][:, hB * 128:(hB + 1) * 128],
                                ppa[:, 512:1024],
                                start=(pst == 0), stop=False,
                            )
                        pending = (pa, st)
                    ppa, pst = pending
                    nc.tensor.matmul(
                        xA[:], vlin_sb[pst][:, hA * 128:(hA + 1) * 128],
                        ppa[:, 0:512], start=False, stop=True,
                    )
                    nc.tensor.matmul(
                        xB[:], vlin_sb[pst][:, hB * 128:(hB + 1) * 128],
                        ppa[:, 512:1024], start=False, stop=True,
                    )
                    # normalize; rows 0:64 = X^T_h, 64:128 = replicated denoms
                    rxA = rxp.tile([64, 512], F32, tag="rxA")
                    nc.vector.reciprocal(rxA[:], xA[64:128, :])
                    nc.vector.tensor_mul(XhatT[c][0:64, tsl], xA[0:64, :], rxA[:])
                    rxB = rxp.tile([64, 512], F32, tag="rxB")
                    nc.vector.reciprocal(rxB[:], xB[64:128, :])
                    nc.vector.tensor_mul(XhatT[c][64:128, tsl], xB[0:64, :], rxB[:])
                    if pair == H // 2 - 1:
                        for tb in range(4 * tcx, 4 * tcx + 4):
                            outlin_block(tb)


def _get_program():
    global _PROGRAM
    if _PROGRAM is None:
        _PROGRAM = _build_core_program()
    return _PROGRAM


def _prep_core_inputs(q, k, v, W0, b0, Wout, bout):
    """Host-side layout prep for one batch element (layout/dtype only)."""
    qTp = np.zeros((D, SQ), BF16)
    qTp[:, 0:S] = np.ascontiguousarray(q.T).astype(BF16)
    kTp = np.zeros((D, SQ), BF16)
    kTp[:, 0:S] = np.ascontiguousarray(k.T).astype(BF16)
    vTp = np.zeros((D, SPP), BF16)
    vTp[:, 0:S - 4] = np.ascontiguousarray(v[4:].T).astype(BF16)
    return {
        "qT": qTp,
        "kT": kTp,
        "vT": vTp,
        "W0": W0.astype(BF16),
        "Wout": Wout.astype(BF16),
        "b0": b0.reshape(1, D).astype(BF16),
        "bout": bout.reshape(1, D).astype(BF16),
        "zpad": np.zeros((4, 1024), BF16),
    }


def kernel(query, key, value, W0, b0, Wout, bout):
    query = np.asarray(query, np.float32)
    key = np.asarray(key, np.float32)
    value = np.asarray(value, np.float32)
    W0 = np.asarray(W0, np.float32)
    b0 = np.asarray(b0, np.float32)
    Wout = np.asarray(Wout, np.float32)
    bout = np.asarray(bout, np.float32)

    nc = _get_program()
    in_maps = [
        _prep_core_inputs(query[b], key[b], value[b], W0, b0, Wout, bout)
        for b in range(B)
    ]
    res = run_bass_kernel_spmd(nc, in_maps, list(range(N_CORES)))
    return np.stack([res.results[b]["out"] for b in range(B)], axis=0)

